# revision 59
# baseline (speedup 1.0000x reference)
"""DIN-style attention + MLP trunk, Trainium2 Bass kernel, 8-core data parallel.

Shapes (hardcoded): B=32, T=200, TQ=50, E=64, P=128, C=64, U=36.

v3 design (transposed attention layout, cost-model driven; 24.0us vs the
28.8us v2 baseline, TimelineSim):
  * mm1 computes z^T[(q,u), t] = augR_chunk^T @ augL per batch: stationary =
    augR chunks [65, 120] (qu-rows), moving = augL [65, 200] (t).  PE cols
    per batch: 15 chunks x 200 = 3000 (vs 3600 in the [t,qu] layout), and
    the Silu evictions are 94%-partition-dense: ACT (the bottleneck engine,
    1.2GHz, the only engine with sigmoid) runs 12.6us busy vs v2's 15us.
  * The W2 contraction over u runs on PE as banded accumulating matmuls:
    each chunk's Sel band is a [120, 4] slice of the host-built selection
    matrix Sel[(qu-row), q] = W2[u]/c.  wp psum is DVE-memset once; all
    matmuls accumulate with start=False (overlapping 4-col windows).  This
    replaces v2's DVE tensor_tensor + grouped reduce (9us DVE) with ~1k PE
    cols; DVE drops to ~4us.
  * w[t, q] lands directly in [t-part, q-free] orientation (no transpose),
    is evicted per-t-chunk to two bf16 tiles (parallel ACT/DVE copies).
  * Tail latency: for the last pair the interest matmul G is folded into
    trunk layer 1 via V = ub @ W1f[0:64] (precomputed mid-stream):
    x1 = V^T @ w + W1f[64:]^T @ [up;cx], with w's q-columns split 0:30
    (ready before the last silu) / 30:50 (the only post-silu work).  The
    up/cx part and q-0:30 V-matmuls run during the final silu; layer 3 and
    the out-DMA are split per q-half so DMA latency overlaps compute.
  * Schedule is software-pipelined by hand: each batch's first (3-chunk)
    mm1 tile is emitted before the previous batch's trailing wsel matmuls,
    giving zero-gap ACT at batch borders; silu tiles are (3,6,6) chunks.
  * termq (+b1) rides as augR row 64 against the augL ones-row, dice's
    rsqrt scale is the Silu `scale`, W2/c is folded into Sel, BN scales
    into the trunk weights.  DMA is compact (1.35MB/core, b0-first order).
  * psum banks: zp 2x3 (z-chunks, 6 per tile, one strided-AP Silu each) +
    wp 1 (w accumulator) + gx 1 (G/V/trunk rotating) = 8.
"""

from contextlib import ExitStack

import ml_dtypes
import numpy as np

import concourse.bacc as bacc
import concourse.tile as tile
from concourse import mybir
from concourse.bass_utils import run_bass_kernel_spmd

F32 = mybir.dt.float32
BF16 = mybir.dt.bfloat16
BF16NP = ml_dtypes.bfloat16

B, T, TQ, E = 32, 200, 50, 64
P, C = 128, 64
U = 36
NCORES = 8
BL = B // NCORES   # batches per core
QU = TQ * U        # 1800
CH = 120           # qu-rows per mm1 chunk
NCH = QU // CH     # 15 chunks per batch
EPS = 1e-6

TCH = [(0, 128), (128, 72)]

# z-chunk column offsets inside a [128, 1536] (3-bank) psum tile, ordered so
# any prefix of 3 and the full 6 form rectangular strided APs:
#   first 3: stride 512; 6: [2 x stride 200, 3 x stride 512]
SLOT = [0, 512, 1024, 200, 712, 1224]

# (chunk_start, n_chunks) silu tiles per batch: a small 3-chunk first tile
# lets the next batch's first silu fire with zero ACT gap at batch borders
# (the explicit schedule below references these splits directly)
TILES = [(0, 3), (3, 6), (9, 6)]

# consts pack layout (columns of one [128, 1232] bf16 tensor)
CB0 = 0            # trunk weights: w1f k0 | w1f k1 | w2f k0 | w2f k1 | w3f
CH1 = 832          # chunk1 = [up^T[64:128]; cx^T] broadcast over q (200 cols)
CH0 = 1032         # chunk0 = [zeros(interest); up^T[0:64]] broadcast (200)
NCONST = 1232

BW = T + QU        # augLR cols per batch: [augL_b (200) | augR_b (1800)]

_CACHE = {}


def _build_program():
    nc = bacc.Bacc(
        "TRN2", target_bir_lowering=False, debug=False, num_devices=NCORES
    )
    d_augLR = nc.declare_dram_parameter(
        "augLR", [65, BL * BW], BF16, isOutput=False
    )
    d_sel = nc.declare_dram_parameter("sel", [CH, NCH * 4], BF16, isOutput=False)
    d_ubt = nc.declare_dram_parameter("ubt", [128, 2 * BL * E], BF16, isOutput=False)
    d_consts = nc.declare_dram_parameter("consts", [128, NCONST], BF16, isOutput=False)
    d_out = nc.declare_dram_parameter("out", [64, BL * TQ], F32, isOutput=True)

    c_dice = float(1.0 / np.sqrt(1.0 + EPS))

    with tile.TileContext(nc) as tc:
        with ExitStack() as ctx:
            singles = ctx.enter_context(tc.tile_pool(name="singles", bufs=1))
            work = ctx.enter_context(tc.tile_pool(name="work", bufs=2))
            ps_z = ctx.enter_context(tc.tile_pool(name="ps_z", bufs=2, space="PSUM"))
            ps_w = ctx.enter_context(tc.tile_pool(name="ps_w", bufs=1, space="PSUM"))
            ps_g = ctx.enter_context(tc.tile_pool(name="ps_g", bufs=1, space="PSUM"))

            augLR = singles.tile([65, BL * BW], BF16)
            # first DMA: b0's augL + augR chunks 0-2 -> unblocks the first
            # mm1 tile + silu as early as possible
            nc.sync.dma_start(
                out=augLR[:, 0:T + 3 * CH], in_=d_augLR[:, 0:T + 3 * CH]
            )
            nc.sync.dma_start(
                out=augLR[:, T + 3 * CH:T + 9 * CH],
                in_=d_augLR[:, T + 3 * CH:T + 9 * CH],
            )
            nc.sync.dma_start(
                out=augLR[:, T + 9 * CH:BW], in_=d_augLR[:, T + 9 * CH:BW]
            )
            sel = singles.tile([CH, NCH * 4], BF16)
            nc.sync.dma_start(out=sel, in_=d_sel[:])
            nc.sync.dma_start(out=augLR[:, BW:2 * BW], in_=d_augLR[:, BW:2 * BW])
            ubt = singles.tile([128, 2 * BL * E], BF16)
            nc.sync.dma_start(out=ubt, in_=d_ubt[:])
            consts = singles.tile([128, NCONST], BF16)
            nc.sync.dma_start(out=consts, in_=d_consts[:])
            nc.sync.dma_start(
                out=augLR[:, 2 * BW:3 * BW], in_=d_augLR[:, 2 * BW:3 * BW]
            )
            nc.sync.dma_start(
                out=augLR[:, 3 * BW:], in_=d_augLR[:, 3 * BW:]
            )

            w1f_sb = [consts[:, 0:256], consts[:, 256:512]]
            w2f_sb = [consts[:, 512:640], consts[:, 640:768]]
            w3f_sb = consts[:, 768:832]
            chunk1 = consts[:, CH1:CH1 + BL * TQ]
            chunk0 = consts[:, CH0:CH0 + BL * TQ]

            w_tiles = {}
            wps = {}
            # s' for ALL batches lives in one tensor, columns indexed by the
            # GLOBAL chunk id g = 15b + c -- contiguous across batch borders,
            # so silu tiles can span two batches (no border ACT gaps).
            s_all = singles.tile([128, BL * NCH * T], BF16, name="s_all")

            def mm1_tile(g0, ncb):
                """z^T global chunks g0..g0+ncb -> fresh zp tile."""
                zp = ps_z.tile([128, 1536], F32, tag="zp")
                for i in range(ncb):
                    g = g0 + i
                    b, c = g // NCH, g % NCH
                    nc.tensor.matmul(
                        zp[0:CH, SLOT[i]:SLOT[i] + T],
                        augLR[:, b * BW + T + CH * c:b * BW + T + CH * (c + 1)],
                        augLR[:, b * BW:b * BW + T],
                        start=True,
                        stop=True,
                    )
                return zp

            def silu_tile(g0, ncb, zp):
                """One Silu draining ncb z-chunks (ncb in {3, 6})."""
                zin = zp[0:CH, :].rearrange("p (a r) -> p a r", r=512)[:, :, 0:400]
                zin = zin.rearrange("p a (two c) -> p two a c", two=2)
                out = s_all[0:CH, T * g0:T * (g0 + ncb)]
                if ncb == 6:
                    nc.scalar.activation(
                        out.rearrange("p (two a c) -> p two a c", two=2, c=T),
                        zin,
                        mybir.ActivationFunctionType.Silu,
                        scale=c_dice,
                    )
                else:
                    assert ncb == 3
                    nc.scalar.activation(
                        out.rearrange("p (a c) -> p a c", c=T),
                        zin[:, 0],
                        mybir.ActivationFunctionType.Silu,
                        scale=c_dice,
                    )

            def wsel_init(b):
                """Allocate + zero wp for batch b.  Called explicitly ahead
                of the first wsel so the memset clears the DVE queue early
                instead of gating the batch's first wsel matmuls."""
                if b not in wps:
                    wps[b] = ps_w.tile([128, 128], F32, tag="wp", name=f"wp{b}")
                    nc.vector.memset(wps[b][:], 0.0)

            def wsel_tile(b, t0c, ncb, tch_only=None):
                """Accumulate chunks' banded W2-contraction into wp.  Each
                chunk's Sel band is 4 columns wide (its q-range), so matmuls
                are 4-col with overlapping accumulation windows.  wp is
                pre-zeroed by DVE memset and every matmul uses start=False
                (clean accumulate semantics, no bank pending-zero games)."""
                wsel_init(b)
                wp = wps[b]
                for tch, (tc0, tlen) in enumerate(TCH):
                    if tch_only is not None and tch != tch_only:
                        continue
                    for i in range(ncb):
                        c = t0c + i
                        g = NCH * b + c
                        qoff = (10 * c) // 3
                        nc.tensor.matmul(
                            wp[0:tlen, 64 * tch + qoff:64 * tch + qoff + 4],
                            s_all[0:CH, T * g + tc0:T * g + tc0 + tlen],
                            sel[:, 4 * c:4 * (c + 1)],
                            start=False,
                            stop=(tch == 1 and c == NCH - 1),
                            skip_group_check=True,
                        )

            def wevict(b, q0=0, q1=TQ, split=False):
                """Evict wp cols [q0, q1) -> per-t-chunk bf16 tiles (separate
                tiles: no same-tile WAW between the two copies)."""
                if b not in w_tiles:
                    w_tiles[b] = [
                        work.tile([128, 64], BF16, tag=f"wA{b % 2}",
                                  name=f"wA{b}"),
                        work.tile([128, 64], BF16, tag=f"wB{b % 2}",
                                  name=f"wB{b}"),
                    ]
                wp = wps[b]
                nc.vector.tensor_copy(
                    w_tiles[b][0][:, q0:q1], wp[:, q0:q1]
                )
                if split:  # tail: second copy on the idle ACT engine
                    nc.scalar.copy(
                        w_tiles[b][1][0:72, q0:q1], wp[0:72, 64 + q0:64 + q1]
                    )
                else:
                    nc.vector.tensor_copy(
                        w_tiles[b][1][0:72, q0:q1], wp[0:72, 64 + q0:64 + q1]
                    )

            def g_pair(pb):
                """interest for pair (2pb, 2pb+1), pair-packed [128, 50]."""
                gp = ps_g.tile([128, 512], F32, tag="gx")
                for half in range(2):
                    b = 2 * pb + half
                    for tch, (tc0, tlen) in enumerate(TCH):
                        nc.tensor.matmul(
                            gp[64 * half:64 * half + 64, 0:TQ],
                            ubt[0:tlen, tch * BL * E + b * E:tch * BL * E + (b + 1) * E],
                            w_tiles[b][tch][0:tlen, 0:TQ],
                            start=(tch == 0),
                            stop=(tch == 1),
                        )
                pair = (2 * pb, 2 * pb + 1)
                nc.vector.tensor_copy(
                    chunk0[0:64, pair[0] * TQ:(pair[0] + 1) * TQ], gp[0:64, 0:TQ]
                )
                nc.vector.tensor_copy(
                    chunk0[0:64, pair[1] * TQ:(pair[1] + 1) * TQ], gp[64:128, 0:TQ]
                )

            v_sb = {}

            def v_precompute():
                """V = ub @ w1f[0:64] for pair-1 batches (2, 3): folds the
                interest matmul (G) into the trunk's first layer so the tail
                chain skips G + g_fin entirely.  Runs mid-stream (off the
                critical path); stationary is augL rows 0:64.  Both batches
                share one psum tile and one wide DVE eviction per t-chunk."""
                for tch, (tc0, tlen) in enumerate(TCH):
                    v_sb[tch] = singles.tile([128, 512], BF16, name=f"v{tch}")
                    # wp bank is free between wevict(1) and wsel(2,..):
                    # using it here keeps the gx chain (g_pair->trunk0) short
                    vp = ps_w.tile([128, 512], F32, tag="wp")
                    for b in (2, 3):
                        # single start: b3's region rides b2's bank-wide
                        # pending-zero (same trick as wsel_tile)
                        nc.tensor.matmul(
                            vp[0:tlen, (b - 2) * 256:(b - 1) * 256],
                            augLR[0:64, b * BW + tc0:b * BW + tc0 + tlen],
                            consts[0:64, 0:256],
                            start=(b == 2),
                            stop=(b == 3),
                            skip_group_check=True,
                        )
                    nc.vector.tensor_copy(
                        v_sb[tch][0:tlen, :], vp[0:tlen, :]
                    )

            tail_xp = {}

            def trunk_tail_waveA(pb):
                """Tail trunk, early wave (runs during the last silu): G is
                folded into layer 1 via x1 = V^T @ w + W1f[64:]^T @ [up;cx].
                Everything except b_last's q-40:50 w-columns is ready once
                wsel of chunks 0-11 lands, so layer-1 is mostly done before
                the final silu finishes.  psum: mch0 in gx, mch1 in a freed
                zp slot (wp slot still accumulates b_last's wsel)."""
                n0c = 2 * pb * TQ
                cols = slice(n0c, n0c + 2 * TQ)
                for mch in range(2):
                    if mch == 1:
                        xp = ps_z.tile([128, 1536], F32, tag="zp")
                    else:
                        xp = ps_g.tile([128, 512], F32, tag="gx")
                    tail_xp[mch] = xp
                    nc.tensor.matmul(
                        xp[:, 0:2 * TQ],
                        w1f_sb[0][64:128, mch * 128:(mch + 1) * 128],
                        chunk0[64:128, cols], start=True, stop=False,
                    )
                    nc.tensor.matmul(
                        xp[:, 0:2 * TQ],
                        w1f_sb[1][:, mch * 128:(mch + 1) * 128],
                        chunk1[:, cols], start=False, stop=False,
                    )
                    for half in range(2):
                        b = 2 * pb + half
                        qe = 40 if half == 1 else TQ  # b_last: A-cols only
                        for tch, (tc0, tlen) in enumerate(TCH):
                            nc.tensor.matmul(
                                xp[:, half * TQ:half * TQ + qe],
                                v_sb[tch][0:tlen,
                                          half * 256 + mch * 128:
                                          half * 256 + (mch + 1) * 128],
                                w_tiles[b][tch][0:tlen, 0:qe],
                                start=False,
                                stop=False,
                                skip_group_check=True,
                            )

            def trunk_tail_waveB(pb):
                """Tail trunk, late wave: only b_last's w[:, 40:50] columns
                remained; then relu -> layer2 -> layer3 -> out DMAs."""
                n0c = 2 * pb * TQ
                b = 2 * pb + 1
                x1 = []
                for mch in range(2):
                    xp = tail_xp[mch]
                    for tch, (tc0, tlen) in enumerate(TCH):
                        nc.tensor.matmul(
                            xp[:, TQ + 40:2 * TQ],
                            v_sb[tch][0:tlen,
                                      256 + mch * 128:256 + (mch + 1) * 128],
                            w_tiles[b][tch][0:tlen, 40:TQ],
                            start=False,
                            stop=(tch == 1),
                            skip_group_check=True,
                        )
                    x1_t = work.tile([128, 2 * TQ], BF16, tag=f"x1_{mch}")
                    relu_evict(x1_t, xp[:, 0:2 * TQ], mch == 1)
                    x1.append(x1_t)

                xp2 = ps_g.tile([128, 512], F32, tag="gx")
                nc.tensor.matmul(xp2[:, 0:2 * TQ], w2f_sb[0], x1[0],
                                 start=True, stop=False)
                nc.tensor.matmul(xp2[:, 0:2 * TQ], w2f_sb[1], x1[1],
                                 start=False, stop=True)
                # x2 relu + layer 3 + out, pipelined per q-half (h0 on DVE,
                # h1 on ACT): each half's DMA overlaps the other's compute
                x2_t = work.tile([128, 2 * TQ], BF16, tag="x2")
                xp3 = ps_z.tile([64, 1536], F32, tag="zp")
                for h in range(2):
                    relu_evict(
                        x2_t[:, h * TQ:(h + 1) * TQ],
                        xp2[:, h * TQ:(h + 1) * TQ], h == 0,
                    )
                    nc.tensor.matmul(
                        xp3[:, h * TQ:(h + 1) * TQ], w3f_sb,
                        x2_t[:, h * TQ:(h + 1) * TQ], start=(h == 0),
                        stop=(h == 1), skip_group_check=True,
                    )
                    out_t = work.tile([64, TQ], F32, tag=f"outT{h}")
                    relu_evict(out_t, xp3[:, h * TQ:(h + 1) * TQ], h == 0)
                    nc.sync.dma_start(
                        out=d_out[:, n0c + h * TQ:n0c + (h + 1) * TQ],
                        in_=out_t,
                    )

            def relu_evict(out_t, xp_ap, on_act):
                if on_act:
                    nc.scalar.activation(
                        out_t, xp_ap, mybir.ActivationFunctionType.Relu
                    )
                else:
                    nc.vector.tensor_scalar_max(out_t, xp_ap, 0.0)

            def trunk_pair(pb, tail=False):
                """tail=True (last pair): second psum slot (freed wp bank) +
                ACT for half the evictions, parallelizing the stage chain."""
                n0c = 2 * pb * TQ
                cols = slice(n0c, n0c + 2 * TQ)
                x1 = []
                for mch in range(2):
                    if tail and mch == 1:
                        xp = ps_w.tile([128, 128], F32, tag="wp")
                    else:
                        xp = ps_g.tile([128, 512], F32, tag="gx")
                    nc.tensor.matmul(
                        xp[:, 0:2 * TQ],
                        w1f_sb[0][:, mch * 128:(mch + 1) * 128],
                        chunk0[:, cols], start=True, stop=False,
                    )
                    nc.tensor.matmul(
                        xp[:, 0:2 * TQ],
                        w1f_sb[1][:, mch * 128:(mch + 1) * 128],
                        chunk1[:, cols], start=False, stop=True,
                    )
                    x1_t = work.tile([128, 2 * TQ], BF16, tag=f"x1_{mch}")
                    relu_evict(x1_t, xp[:, 0:2 * TQ], tail and mch == 1)
                    x1.append(x1_t)

                xp2 = ps_g.tile([128, 512], F32, tag="gx")
                nc.tensor.matmul(xp2[:, 0:2 * TQ], w2f_sb[0], x1[0],
                                 start=True, stop=False)
                nc.tensor.matmul(xp2[:, 0:2 * TQ], w2f_sb[1], x1[1],
                                 start=False, stop=True)
                x2_t = work.tile([128, 2 * TQ], BF16, tag="x2")
                relu_evict(x2_t, xp2[:, 0:2 * TQ], False)

                if tail:
                    xp3 = ps_w.tile([64, 128], F32, tag="wp")
                else:
                    xp3 = ps_g.tile([64, 512], F32, tag="gx")
                nc.tensor.matmul(xp3[:, 0:2 * TQ], w3f_sb, x2_t,
                                 start=True, stop=True)
                out_t = work.tile([64, 2 * TQ], F32, tag="outT")
                relu_evict(out_t, xp3[:, 0:2 * TQ], tail)
                nc.sync.dma_start(out=d_out[:, cols], in_=out_t)

            def tile_step(g0, ncb):
                zp = mm1_tile(g0, ncb)
                silu_tile(g0, ncb, zp)

            # Software-pipelined schedule over GLOBAL chunk tiles
            # (3, 6x9, 3): every 6-chunk tile's mm1s fit inside the previous
            # silu's 1185ns window, so ACT runs gap-free end to end
            # (batch borders are crossed inside tiles T5=g27-32, etc.).
            tile_step(0, 3)    # b0 c0-2
            tile_step(3, 6)    # b0 c3-8
            wsel_tile(0, 0, 3)
            tile_step(9, 6)    # b0 c9-14
            wsel_tile(0, 3, 6)
            tile_step(15, 6)   # b1 c0-5
            wsel_tile(0, 9, 6)
            wevict(0)
            tile_step(21, 6)   # b1 c6-11
            wsel_tile(1, 0, 6)
            tile_step(27, 6)   # b1 c12-14 | b2 c0-2
            wsel_tile(1, 6, 6)
            tile_step(33, 6)   # b2 c3-8
            wsel_tile(1, 12, 3)
            wevict(1)
            g_pair(0)
            v_precompute()
            wsel_tile(2, 0, 3)
            trunk_pair(0)
            tile_step(39, 6)   # b2 c9-14
            wsel_tile(2, 3, 6)
            tile_step(45, 6)   # b3 c0-5
            wsel_tile(2, 9, 6)
            wevict(2)
            wsel_init(3)
            tile_step(51, 6)   # b3 c6-11
            wsel_tile(3, 0, 6)
            tile_step(57, 3)   # b3 c12-14
            wsel_tile(3, 6, 6)
            wevict(3, 0, 40)
            wsel_tile(3, 12, 3)
            trunk_tail_waveA(1)
            wevict(3, 40, TQ)
            trunk_tail_waveB(1)

    nc.compile()
    return nc


def _prepare_maps(inputs):
    f = lambda k: np.ascontiguousarray(np.asarray(inputs[k], dtype=np.float32))
    W1, W2 = f("W1"), f("W2")
    b1 = f("b1")
    Wm1, Wm2, Wm3 = f("Wm1"), f("Wm2"), f("Wm3")

    A = W1[0:64] + W1[128:192]     # q rows + (q-k) rows
    Bm = W1[64:128] - W1[128:192]  # k rows - (q-k) rows
    D = W1[192:256]                # (q*k) rows
    c = 1.0 / np.sqrt(1.0 + EPS)   # dice rsqrt(var+eps) with var=1
    cb = 1.0 / np.sqrt(1.0 + EPS)  # BN identity scale

    w1f = cb * Wm1
    w2f = cb * Wm2
    w3f = cb * Wm3
    cB = np.concatenate(
        [w1f[0:128], w1f[128:256], w2f[0:128], w2f[128:256], w3f], axis=1
    )  # (128, 832)

    # Banded Sel: chunk c's qu-rows touch exactly 4 q's starting at
    # qoff(c) = (10c)//3; sel4[p, 4c + j] = W2[u]/c for q == qoff(c)+j
    selm = np.zeros((CH, NCH * 4), np.float32)
    for cc in range(NCH):
        qoff = (10 * cc) // 3
        for p in range(CH):
            r = 120 * cc + p
            j = r // U - qoff
            selm[p, 4 * cc + j] = W2[r % U, 0] / c

    ub = f("user_behavior")        # (B, T, E)
    it = f("items")                # (B, TQ, E)
    up = f("user_profile")         # (B, P)
    cx = f("context")              # (B, C)

    in_maps = []
    for i in range(NCORES):
        s = slice(i * BL, (i + 1) * BL)
        ub_i, it_i = ub[s], it[s]

        augLR = np.empty((65, BL * BW), np.float32)
        itt = it_i.transpose(0, 2, 1)  # (BL, E, TQ)
        mprime = (
            itt[:, :, :, None] * D[None, :, None, :]
            + A[None, :, None, :]
        ).reshape(BL, E, QU)
        termq = (
            np.einsum("bqe,eu->bqu", it_i, Bm) + b1[None, None, :]
        ).reshape(BL, QU)
        for b in range(BL):
            augLR[0:64, b * BW:b * BW + T] = ub_i[b].T
            augLR[64, b * BW:b * BW + T] = 1.0
            cols = slice(b * BW + T, (b + 1) * BW)
            augLR[0:64, cols] = mprime[b]
            augLR[64, cols] = termq[b]

        ubt = np.zeros((128, 2 * BL * E), np.float32)
        for b in range(BL):
            ubt[0:128, b * E:(b + 1) * E] = ub_i[b, 0:128]
            ubt[0:72, BL * E + b * E:BL * E + (b + 1) * E] = ub_i[b, 128:200]

        consts = np.zeros((128, NCONST), np.float32)
        consts[:, 0:832] = cB
        for b in range(BL):
            cols = slice(CH1 + b * TQ, CH1 + (b + 1) * TQ)
            consts[0:64, cols] = up[s][b, 64:128, None]
            consts[64:128, cols] = cx[s][b, :, None]
            cols = slice(CH0 + b * TQ, CH0 + (b + 1) * TQ)
            consts[64:128, cols] = up[s][b, 0:64, None]

        in_maps.append({
            "augLR": np.ascontiguousarray(augLR.astype(BF16NP)),
            "sel": np.ascontiguousarray(selm.astype(BF16NP)),
            "ubt": np.ascontiguousarray(ubt.astype(BF16NP)),
            "consts": np.ascontiguousarray(consts.astype(BF16NP)),
        })
    return in_maps


def run(inputs, trace=False):
    if "nc" not in _CACHE:
        _CACHE["nc"] = _build_program()
    nc = _CACHE["nc"]
    in_maps = _prepare_maps(inputs)
    res = run_bass_kernel_spmd(nc, in_maps, list(range(NCORES)), trace=trace)
    out = np.empty((B, TQ, 64), dtype=np.float32)
    for i in range(NCORES):
        out[i * BL:(i + 1) * BL] = (
            res.results[i]["out"].T.reshape(BL, TQ, 64)
        )
    return out, res


def kernel(**inputs):
    out, _ = run(inputs, trace=False)
    return out


# revision 60
# speedup vs baseline: 1.0077x; 1.0077x over previous
"""DIN-style attention + MLP trunk, Trainium2 Bass kernel, 8-core data parallel.

Shapes (hardcoded): B=32, T=200, TQ=50, E=64, P=128, C=64, U=36.

v3 design (transposed attention layout, cost-model driven; 24.0us vs the
28.8us v2 baseline, TimelineSim):
  * mm1 computes z^T[(q,u), t] = augR_chunk^T @ augL per batch: stationary =
    augR chunks [65, 120] (qu-rows), moving = augL [65, 200] (t).  PE cols
    per batch: 15 chunks x 200 = 3000 (vs 3600 in the [t,qu] layout), and
    the Silu evictions are 94%-partition-dense: ACT (the bottleneck engine,
    1.2GHz, the only engine with sigmoid) runs 12.6us busy vs v2's 15us.
  * The W2 contraction over u runs on PE as banded accumulating matmuls:
    each chunk's Sel band is a [120, 4] slice of the host-built selection
    matrix Sel[(qu-row), q] = W2[u]/c.  wp psum is DVE-memset once; all
    matmuls accumulate with start=False (overlapping 4-col windows).  This
    replaces v2's DVE tensor_tensor + grouped reduce (9us DVE) with ~1k PE
    cols; DVE drops to ~4us.
  * w[t, q] lands directly in [t-part, q-free] orientation (no transpose),
    is evicted per-t-chunk to two bf16 tiles (parallel ACT/DVE copies).
  * Tail latency: for the last pair the interest matmul G is folded into
    trunk layer 1 via V = ub @ W1f[0:64] (precomputed mid-stream):
    x1 = V^T @ w + W1f[64:]^T @ [up;cx], with w's q-columns split 0:30
    (ready before the last silu) / 30:50 (the only post-silu work).  The
    up/cx part and q-0:30 V-matmuls run during the final silu; layer 3 and
    the out-DMA are split per q-half so DMA latency overlaps compute.
  * Schedule is software-pipelined by hand: each batch's first (3-chunk)
    mm1 tile is emitted before the previous batch's trailing wsel matmuls,
    giving zero-gap ACT at batch borders; silu tiles are (3,6,6) chunks.
  * termq (+b1) rides as augR row 64 against the augL ones-row, dice's
    rsqrt scale is the Silu `scale`, W2/c is folded into Sel, BN scales
    into the trunk weights.  DMA is compact (1.35MB/core, b0-first order).
  * psum banks: zp 2x3 (z-chunks, 6 per tile, one strided-AP Silu each) +
    wp 1 (w accumulator) + gx 1 (G/V/trunk rotating) = 8.
"""

from contextlib import ExitStack

import ml_dtypes
import numpy as np

import concourse.bacc as bacc
import concourse.tile as tile
from concourse import mybir
from concourse.bass_utils import run_bass_kernel_spmd

F32 = mybir.dt.float32
BF16 = mybir.dt.bfloat16
BF16NP = ml_dtypes.bfloat16

B, T, TQ, E = 32, 200, 50, 64
P, C = 128, 64
U = 36
NCORES = 8
BL = B // NCORES   # batches per core
QU = TQ * U        # 1800
CH = 120           # qu-rows per mm1 chunk
NCH = QU // CH     # 15 chunks per batch
EPS = 1e-6

TCH = [(0, 128), (128, 72)]

# z-chunk column offsets inside a [128, 1536] (3-bank) psum tile, ordered so
# any prefix of 3 and the full 6 form rectangular strided APs:
#   first 3: stride 512; 6: [2 x stride 200, 3 x stride 512]
SLOT = [0, 512, 1024, 200, 712, 1224]

# (chunk_start, n_chunks) silu tiles per batch: a small 3-chunk first tile
# lets the next batch's first silu fire with zero ACT gap at batch borders
# (the explicit schedule below references these splits directly)
TILES = [(0, 3), (3, 6), (9, 6)]

# consts pack layout (columns of one [128, 1232] bf16 tensor)
CB0 = 0            # trunk weights: w1f k0 | w1f k1 | w2f k0 | w2f k1 | w3f
CH1 = 832          # chunk1 = [up^T[64:128]; cx^T] broadcast over q (200 cols)
CH0 = 1032         # chunk0 = [zeros(interest); up^T[0:64]] broadcast (200)
NCONST = 1232

BW = T + QU        # augLR cols per batch: [augL_b (200) | augR_b (1800)]

_CACHE = {}


def _build_program():
    nc = bacc.Bacc(
        "TRN2", target_bir_lowering=False, debug=False, num_devices=NCORES
    )
    d_augLR = nc.declare_dram_parameter(
        "augLR", [65, BL * BW], BF16, isOutput=False
    )
    d_sel = nc.declare_dram_parameter("sel", [CH, NCH * 4], BF16, isOutput=False)
    d_ubt = nc.declare_dram_parameter("ubt", [128, 2 * BL * E], BF16, isOutput=False)
    d_consts = nc.declare_dram_parameter("consts", [128, NCONST], BF16, isOutput=False)
    d_out = nc.declare_dram_parameter("out", [64, BL * TQ], F32, isOutput=True)

    c_dice = float(1.0 / np.sqrt(1.0 + EPS))

    with tile.TileContext(nc) as tc:
        with ExitStack() as ctx:
            singles = ctx.enter_context(tc.tile_pool(name="singles", bufs=1))
            work = ctx.enter_context(tc.tile_pool(name="work", bufs=2))
            ps_z = ctx.enter_context(tc.tile_pool(name="ps_z", bufs=2, space="PSUM"))
            ps_w = ctx.enter_context(tc.tile_pool(name="ps_w", bufs=1, space="PSUM"))
            ps_g = ctx.enter_context(tc.tile_pool(name="ps_g", bufs=1, space="PSUM"))

            augLR = singles.tile([65, BL * BW], BF16)
            # first DMA: b0's augL + augR chunks 0-2 -> unblocks the first
            # mm1 tile + silu as early as possible
            nc.sync.dma_start(
                out=augLR[:, 0:T + 3 * CH], in_=d_augLR[:, 0:T + 3 * CH]
            )
            nc.sync.dma_start(
                out=augLR[:, T + 3 * CH:T + 9 * CH],
                in_=d_augLR[:, T + 3 * CH:T + 9 * CH],
            )
            nc.sync.dma_start(
                out=augLR[:, T + 9 * CH:BW], in_=d_augLR[:, T + 9 * CH:BW]
            )
            sel = singles.tile([CH, NCH * 4], BF16)
            nc.sync.dma_start(out=sel, in_=d_sel[:])
            nc.sync.dma_start(out=augLR[:, BW:2 * BW], in_=d_augLR[:, BW:2 * BW])
            ubt = singles.tile([128, 2 * BL * E], BF16)
            nc.sync.dma_start(out=ubt, in_=d_ubt[:])
            consts = singles.tile([128, NCONST], BF16)
            nc.sync.dma_start(out=consts, in_=d_consts[:])
            nc.sync.dma_start(
                out=augLR[:, 2 * BW:3 * BW], in_=d_augLR[:, 2 * BW:3 * BW]
            )
            nc.sync.dma_start(
                out=augLR[:, 3 * BW:], in_=d_augLR[:, 3 * BW:]
            )

            w1f_sb = [consts[:, 0:256], consts[:, 256:512]]
            w2f_sb = [consts[:, 512:640], consts[:, 640:768]]
            w3f_sb = consts[:, 768:832]
            chunk1 = consts[:, CH1:CH1 + BL * TQ]
            chunk0 = consts[:, CH0:CH0 + BL * TQ]

            w_tiles = {}
            wps = {}
            # s' for ALL batches lives in one tensor, columns indexed by the
            # GLOBAL chunk id g = 15b + c -- contiguous across batch borders,
            # so silu tiles can span two batches (no border ACT gaps).
            s_all = singles.tile([128, BL * NCH * T], BF16, name="s_all")

            def mm1_tile(g0, ncb):
                """z^T global chunks g0..g0+ncb -> fresh zp tile."""
                zp = ps_z.tile([128, 1536], F32, tag="zp")
                for i in range(ncb):
                    g = g0 + i
                    b, c = g // NCH, g % NCH
                    nc.tensor.matmul(
                        zp[0:CH, SLOT[i]:SLOT[i] + T],
                        augLR[:, b * BW + T + CH * c:b * BW + T + CH * (c + 1)],
                        augLR[:, b * BW:b * BW + T],
                        start=True,
                        stop=True,
                    )
                return zp

            def silu_tile(g0, ncb, zp):
                """One Silu draining ncb z-chunks (ncb in {3, 6})."""
                zin = zp[0:CH, :].rearrange("p (a r) -> p a r", r=512)[:, :, 0:400]
                zin = zin.rearrange("p a (two c) -> p two a c", two=2)
                out = s_all[0:CH, T * g0:T * (g0 + ncb)]
                if ncb == 6:
                    nc.scalar.activation(
                        out.rearrange("p (two a c) -> p two a c", two=2, c=T),
                        zin,
                        mybir.ActivationFunctionType.Silu,
                        scale=c_dice,
                    )
                else:
                    assert ncb == 3
                    nc.scalar.activation(
                        out.rearrange("p (a c) -> p a c", c=T),
                        zin[:, 0],
                        mybir.ActivationFunctionType.Silu,
                        scale=c_dice,
                    )

            def wsel_init(b):
                """Allocate + zero wp for batch b.  Called explicitly ahead
                of the first wsel so the memset clears the DVE queue early
                instead of gating the batch's first wsel matmuls."""
                if b not in wps:
                    wps[b] = ps_w.tile([128, 128], F32, tag="wp", name=f"wp{b}")
                    nc.vector.memset(wps[b][:], 0.0)

            def wsel_tile(b, t0c, ncb, tch_only=None):
                """Accumulate chunks' banded W2-contraction into wp.  Each
                chunk's Sel band is 4 columns wide (its q-range), so matmuls
                are 4-col with overlapping accumulation windows.  wp is
                pre-zeroed by DVE memset and every matmul uses start=False
                (clean accumulate semantics, no bank pending-zero games)."""
                wsel_init(b)
                wp = wps[b]
                for tch, (tc0, tlen) in enumerate(TCH):
                    if tch_only is not None and tch != tch_only:
                        continue
                    for i in range(ncb):
                        c = t0c + i
                        g = NCH * b + c
                        qoff = (10 * c) // 3
                        nc.tensor.matmul(
                            wp[0:tlen, 64 * tch + qoff:64 * tch + qoff + 4],
                            s_all[0:CH, T * g + tc0:T * g + tc0 + tlen],
                            sel[:, 4 * c:4 * (c + 1)],
                            start=False,
                            stop=(tch == 1 and c == NCH - 1),
                            skip_group_check=True,
                        )

            def wevict(b, q0=0, q1=TQ, split=False):
                """Evict wp cols [q0, q1) -> per-t-chunk bf16 tiles (separate
                tiles: no same-tile WAW between the two copies)."""
                if b not in w_tiles:
                    w_tiles[b] = [
                        work.tile([128, 64], BF16, tag=f"wA{b % 2}",
                                  name=f"wA{b}"),
                        work.tile([128, 64], BF16, tag=f"wB{b % 2}",
                                  name=f"wB{b}"),
                    ]
                wp = wps[b]
                nc.vector.tensor_copy(
                    w_tiles[b][0][:, q0:q1], wp[:, q0:q1]
                )
                if split:  # tail: second copy on the idle ACT engine
                    nc.scalar.copy(
                        w_tiles[b][1][0:72, q0:q1], wp[0:72, 64 + q0:64 + q1]
                    )
                else:
                    nc.vector.tensor_copy(
                        w_tiles[b][1][0:72, q0:q1], wp[0:72, 64 + q0:64 + q1]
                    )

            def g_pair(pb):
                """interest for pair (2pb, 2pb+1), pair-packed [128, 50]."""
                gp = ps_g.tile([128, 512], F32, tag="gx")
                for half in range(2):
                    b = 2 * pb + half
                    for tch, (tc0, tlen) in enumerate(TCH):
                        nc.tensor.matmul(
                            gp[64 * half:64 * half + 64, 0:TQ],
                            ubt[0:tlen, tch * BL * E + b * E:tch * BL * E + (b + 1) * E],
                            w_tiles[b][tch][0:tlen, 0:TQ],
                            start=(tch == 0),
                            stop=(tch == 1),
                        )
                pair = (2 * pb, 2 * pb + 1)
                nc.vector.tensor_copy(
                    chunk0[0:64, pair[0] * TQ:(pair[0] + 1) * TQ], gp[0:64, 0:TQ]
                )
                nc.vector.tensor_copy(
                    chunk0[0:64, pair[1] * TQ:(pair[1] + 1) * TQ], gp[64:128, 0:TQ]
                )

            v_sb = {}

            def v_precompute():
                """V = ub @ w1f[0:64] for pair-1 batches (2, 3): folds the
                interest matmul (G) into the trunk's first layer so the tail
                chain skips G + g_fin entirely.  Runs mid-stream (off the
                critical path); stationary is augL rows 0:64.  Both batches
                share one psum tile and one wide DVE eviction per t-chunk."""
                for tch, (tc0, tlen) in enumerate(TCH):
                    v_sb[tch] = singles.tile([128, 512], BF16, name=f"v{tch}")
                    # wp bank is free between wevict(1) and wsel(2,..):
                    # using it here keeps the gx chain (g_pair->trunk0) short
                    vp = ps_w.tile([128, 512], F32, tag="wp")
                    for b in (2, 3):
                        # single start: b3's region rides b2's bank-wide
                        # pending-zero (same trick as wsel_tile)
                        nc.tensor.matmul(
                            vp[0:tlen, (b - 2) * 256:(b - 1) * 256],
                            augLR[0:64, b * BW + tc0:b * BW + tc0 + tlen],
                            consts[0:64, 0:256],
                            start=(b == 2),
                            stop=(b == 3),
                            skip_group_check=True,
                        )
                    nc.vector.tensor_copy(
                        v_sb[tch][0:tlen, :], vp[0:tlen, :]
                    )

            tail_xp = {}

            def trunk_tail_waveA(pb):
                """Tail trunk, early wave (runs during the last silu): G is
                folded into layer 1 via x1 = V^T @ w + W1f[64:]^T @ [up;cx].
                Everything except b_last's q-40:50 w-columns is ready once
                wsel of chunks 0-11 lands, so layer-1 is mostly done before
                the final silu finishes.  psum: mch0 in gx, mch1 in a freed
                zp slot (wp slot still accumulates b_last's wsel)."""
                n0c = 2 * pb * TQ
                cols = slice(n0c, n0c + 2 * TQ)
                for mch in range(2):
                    if mch == 1:
                        xp = ps_z.tile([128, 1536], F32, tag="zp")
                    else:
                        xp = ps_g.tile([128, 512], F32, tag="gx")
                    tail_xp[mch] = xp
                    nc.tensor.matmul(
                        xp[:, 0:2 * TQ],
                        w1f_sb[0][64:128, mch * 128:(mch + 1) * 128],
                        chunk0[64:128, cols], start=True, stop=False,
                    )
                    nc.tensor.matmul(
                        xp[:, 0:2 * TQ],
                        w1f_sb[1][:, mch * 128:(mch + 1) * 128],
                        chunk1[:, cols], start=False, stop=False,
                    )
                    for half in range(2):
                        b = 2 * pb + half
                        qe = 40 if half == 1 else TQ  # b_last: A-cols only
                        for tch, (tc0, tlen) in enumerate(TCH):
                            nc.tensor.matmul(
                                xp[:, half * TQ:half * TQ + qe],
                                v_sb[tch][0:tlen,
                                          half * 256 + mch * 128:
                                          half * 256 + (mch + 1) * 128],
                                w_tiles[b][tch][0:tlen, 0:qe],
                                start=False,
                                stop=False,
                                skip_group_check=True,
                            )

            def trunk_tail_waveB(pb):
                """Tail trunk, late wave: only b_last's w[:, 40:50] columns
                remained; then relu -> layer2 -> layer3 -> out DMAs."""
                n0c = 2 * pb * TQ
                b = 2 * pb + 1
                x1 = []
                for mch in range(2):
                    xp = tail_xp[mch]
                    for tch, (tc0, tlen) in enumerate(TCH):
                        nc.tensor.matmul(
                            xp[:, TQ + 40:2 * TQ],
                            v_sb[tch][0:tlen,
                                      256 + mch * 128:256 + (mch + 1) * 128],
                            w_tiles[b][tch][0:tlen, 40:TQ],
                            start=False,
                            stop=(tch == 1),
                            skip_group_check=True,
                        )
                    x1_t = work.tile([128, 2 * TQ], BF16, tag=f"x1_{mch}")
                    relu_evict(x1_t, xp[:, 0:2 * TQ], mch == 0)
                    x1.append(x1_t)

                xp2 = ps_g.tile([128, 512], F32, tag="gx")
                nc.tensor.matmul(xp2[:, 0:2 * TQ], w2f_sb[0], x1[0],
                                 start=True, stop=False)
                nc.tensor.matmul(xp2[:, 0:2 * TQ], w2f_sb[1], x1[1],
                                 start=False, stop=True)
                # x2 relu + layer 3 + out, pipelined per q-half (h0 on DVE,
                # h1 on ACT): each half's DMA overlaps the other's compute
                x2_t = work.tile([128, 2 * TQ], BF16, tag="x2")
                xp3 = ps_z.tile([64, 1536], F32, tag="zp")
                for h in range(2):
                    relu_evict(
                        x2_t[:, h * TQ:(h + 1) * TQ],
                        xp2[:, h * TQ:(h + 1) * TQ], h == 1,
                    )
                    nc.tensor.matmul(
                        xp3[:, h * TQ:(h + 1) * TQ], w3f_sb,
                        x2_t[:, h * TQ:(h + 1) * TQ], start=(h == 0),
                        stop=(h == 1), skip_group_check=True,
                    )
                    out_t = work.tile([64, TQ], F32, tag=f"outT{h}")
                    relu_evict(out_t, xp3[:, h * TQ:(h + 1) * TQ], h == 1)
                    nc.sync.dma_start(
                        out=d_out[:, n0c + h * TQ:n0c + (h + 1) * TQ],
                        in_=out_t,
                    )

            def relu_evict(out_t, xp_ap, on_act):
                if on_act:
                    nc.scalar.activation(
                        out_t, xp_ap, mybir.ActivationFunctionType.Relu
                    )
                else:
                    nc.vector.tensor_scalar_max(out_t, xp_ap, 0.0)

            def trunk_pair(pb, tail=False):
                """tail=True (last pair): second psum slot (freed wp bank) +
                ACT for half the evictions, parallelizing the stage chain."""
                n0c = 2 * pb * TQ
                cols = slice(n0c, n0c + 2 * TQ)
                x1 = []
                for mch in range(2):
                    if tail and mch == 1:
                        xp = ps_w.tile([128, 128], F32, tag="wp")
                    else:
                        xp = ps_g.tile([128, 512], F32, tag="gx")
                    nc.tensor.matmul(
                        xp[:, 0:2 * TQ],
                        w1f_sb[0][:, mch * 128:(mch + 1) * 128],
                        chunk0[:, cols], start=True, stop=False,
                    )
                    nc.tensor.matmul(
                        xp[:, 0:2 * TQ],
                        w1f_sb[1][:, mch * 128:(mch + 1) * 128],
                        chunk1[:, cols], start=False, stop=True,
                    )
                    x1_t = work.tile([128, 2 * TQ], BF16, tag=f"x1_{mch}")
                    relu_evict(x1_t, xp[:, 0:2 * TQ], tail and mch == 1)
                    x1.append(x1_t)

                xp2 = ps_g.tile([128, 512], F32, tag="gx")
                nc.tensor.matmul(xp2[:, 0:2 * TQ], w2f_sb[0], x1[0],
                                 start=True, stop=False)
                nc.tensor.matmul(xp2[:, 0:2 * TQ], w2f_sb[1], x1[1],
                                 start=False, stop=True)
                x2_t = work.tile([128, 2 * TQ], BF16, tag="x2")
                relu_evict(x2_t, xp2[:, 0:2 * TQ], False)

                if tail:
                    xp3 = ps_w.tile([64, 128], F32, tag="wp")
                else:
                    xp3 = ps_g.tile([64, 512], F32, tag="gx")
                nc.tensor.matmul(xp3[:, 0:2 * TQ], w3f_sb, x2_t,
                                 start=True, stop=True)
                out_t = work.tile([64, 2 * TQ], F32, tag="outT")
                relu_evict(out_t, xp3[:, 0:2 * TQ], tail)
                nc.sync.dma_start(out=d_out[:, cols], in_=out_t)

            def tile_step(g0, ncb):
                zp = mm1_tile(g0, ncb)
                silu_tile(g0, ncb, zp)

            # Software-pipelined schedule over GLOBAL chunk tiles
            # (3, 6x9, 3): every 6-chunk tile's mm1s fit inside the previous
            # silu's 1185ns window, so ACT runs gap-free end to end
            # (batch borders are crossed inside tiles T5=g27-32, etc.).
            tile_step(0, 3)    # b0 c0-2
            tile_step(3, 6)    # b0 c3-8
            wsel_tile(0, 0, 3)
            tile_step(9, 6)    # b0 c9-14
            wsel_tile(0, 3, 6)
            tile_step(15, 6)   # b1 c0-5
            wsel_tile(0, 9, 6)
            wevict(0)
            tile_step(21, 6)   # b1 c6-11
            wsel_tile(1, 0, 6)
            tile_step(27, 6)   # b1 c12-14 | b2 c0-2
            wsel_tile(1, 6, 6)
            tile_step(33, 6)   # b2 c3-8
            wsel_tile(1, 12, 3)
            wevict(1)
            g_pair(0)
            v_precompute()
            wsel_tile(2, 0, 3)
            trunk_pair(0)
            tile_step(39, 6)   # b2 c9-14
            wsel_tile(2, 3, 6)
            tile_step(45, 6)   # b3 c0-5
            wsel_tile(2, 9, 6)
            wevict(2)
            wsel_init(3)
            tile_step(51, 6)   # b3 c6-11
            wsel_tile(3, 0, 6)
            tile_step(57, 3)   # b3 c12-14
            wsel_tile(3, 6, 6)
            wevict(3, 0, 40)
            wsel_tile(3, 12, 3)
            trunk_tail_waveA(1)
            wevict(3, 40, TQ)
            trunk_tail_waveB(1)

    nc.compile()
    return nc


def _prepare_maps(inputs):
    f = lambda k: np.ascontiguousarray(np.asarray(inputs[k], dtype=np.float32))
    W1, W2 = f("W1"), f("W2")
    b1 = f("b1")
    Wm1, Wm2, Wm3 = f("Wm1"), f("Wm2"), f("Wm3")

    A = W1[0:64] + W1[128:192]     # q rows + (q-k) rows
    Bm = W1[64:128] - W1[128:192]  # k rows - (q-k) rows
    D = W1[192:256]                # (q*k) rows
    c = 1.0 / np.sqrt(1.0 + EPS)   # dice rsqrt(var+eps) with var=1
    cb = 1.0 / np.sqrt(1.0 + EPS)  # BN identity scale

    w1f = cb * Wm1
    w2f = cb * Wm2
    w3f = cb * Wm3
    cB = np.concatenate(
        [w1f[0:128], w1f[128:256], w2f[0:128], w2f[128:256], w3f], axis=1
    )  # (128, 832)

    # Banded Sel: chunk c's qu-rows touch exactly 4 q's starting at
    # qoff(c) = (10c)//3; sel4[p, 4c + j] = W2[u]/c for q == qoff(c)+j
    selm = np.zeros((CH, NCH * 4), np.float32)
    for cc in range(NCH):
        qoff = (10 * cc) // 3
        for p in range(CH):
            r = 120 * cc + p
            j = r // U - qoff
            selm[p, 4 * cc + j] = W2[r % U, 0] / c

    ub = f("user_behavior")        # (B, T, E)
    it = f("items")                # (B, TQ, E)
    up = f("user_profile")         # (B, P)
    cx = f("context")              # (B, C)

    in_maps = []
    for i in range(NCORES):
        s = slice(i * BL, (i + 1) * BL)
        ub_i, it_i = ub[s], it[s]

        augLR = np.empty((65, BL * BW), np.float32)
        itt = it_i.transpose(0, 2, 1)  # (BL, E, TQ)
        mprime = (
            itt[:, :, :, None] * D[None, :, None, :]
            + A[None, :, None, :]
        ).reshape(BL, E, QU)
        termq = (
            np.einsum("bqe,eu->bqu", it_i, Bm) + b1[None, None, :]
        ).reshape(BL, QU)
        for b in range(BL):
            augLR[0:64, b * BW:b * BW + T] = ub_i[b].T
            augLR[64, b * BW:b * BW + T] = 1.0
            cols = slice(b * BW + T, (b + 1) * BW)
            augLR[0:64, cols] = mprime[b]
            augLR[64, cols] = termq[b]

        ubt = np.zeros((128, 2 * BL * E), np.float32)
        for b in range(BL):
            ubt[0:128, b * E:(b + 1) * E] = ub_i[b, 0:128]
            ubt[0:72, BL * E + b * E:BL * E + (b + 1) * E] = ub_i[b, 128:200]

        consts = np.zeros((128, NCONST), np.float32)
        consts[:, 0:832] = cB
        for b in range(BL):
            cols = slice(CH1 + b * TQ, CH1 + (b + 1) * TQ)
            consts[0:64, cols] = up[s][b, 64:128, None]
            consts[64:128, cols] = cx[s][b, :, None]
            cols = slice(CH0 + b * TQ, CH0 + (b + 1) * TQ)
            consts[64:128, cols] = up[s][b, 0:64, None]

        in_maps.append({
            "augLR": np.ascontiguousarray(augLR.astype(BF16NP)),
            "sel": np.ascontiguousarray(selm.astype(BF16NP)),
            "ubt": np.ascontiguousarray(ubt.astype(BF16NP)),
            "consts": np.ascontiguousarray(consts.astype(BF16NP)),
        })
    return in_maps


def run(inputs, trace=False):
    if "nc" not in _CACHE:
        _CACHE["nc"] = _build_program()
    nc = _CACHE["nc"]
    in_maps = _prepare_maps(inputs)
    res = run_bass_kernel_spmd(nc, in_maps, list(range(NCORES)), trace=trace)
    out = np.empty((B, TQ, 64), dtype=np.float32)
    for i in range(NCORES):
        out[i * BL:(i + 1) * BL] = (
            res.results[i]["out"].T.reshape(BL, TQ, 64)
        )
    return out, res


def kernel(**inputs):
    out, _ = run(inputs, trace=False)
    return out


# revision 61
# speedup vs baseline: 1.0094x; 1.0016x over previous
"""DIN-style attention + MLP trunk, Trainium2 Bass kernel, 8-core data parallel.

Shapes (hardcoded): B=32, T=200, TQ=50, E=64, P=128, C=64, U=36.

v3 design (transposed attention layout, cost-model driven; 24.0us vs the
28.8us v2 baseline, TimelineSim):
  * mm1 computes z^T[(q,u), t] = augR_chunk^T @ augL per batch: stationary =
    augR chunks [65, 120] (qu-rows), moving = augL [65, 200] (t).  PE cols
    per batch: 15 chunks x 200 = 3000 (vs 3600 in the [t,qu] layout), and
    the Silu evictions are 94%-partition-dense: ACT (the bottleneck engine,
    1.2GHz, the only engine with sigmoid) runs 12.6us busy vs v2's 15us.
  * The W2 contraction over u runs on PE as banded accumulating matmuls:
    each chunk's Sel band is a [120, 4] slice of the host-built selection
    matrix Sel[(qu-row), q] = W2[u]/c.  wp psum is DVE-memset once; all
    matmuls accumulate with start=False (overlapping 4-col windows).  This
    replaces v2's DVE tensor_tensor + grouped reduce (9us DVE) with ~1k PE
    cols; DVE drops to ~4us.
  * w[t, q] lands directly in [t-part, q-free] orientation (no transpose),
    is evicted per-t-chunk to two bf16 tiles (parallel ACT/DVE copies).
  * Tail latency: for the last pair the interest matmul G is folded into
    trunk layer 1 via V = ub @ W1f[0:64] (precomputed mid-stream):
    x1 = V^T @ w + W1f[64:]^T @ [up;cx], with w's q-columns split 0:30
    (ready before the last silu) / 30:50 (the only post-silu work).  The
    up/cx part and q-0:30 V-matmuls run during the final silu; layer 3 and
    the out-DMA are split per q-half so DMA latency overlaps compute.
  * Schedule is software-pipelined by hand: each batch's first (3-chunk)
    mm1 tile is emitted before the previous batch's trailing wsel matmuls,
    giving zero-gap ACT at batch borders; silu tiles are (3,6,6) chunks.
  * termq (+b1) rides as augR row 64 against the augL ones-row, dice's
    rsqrt scale is the Silu `scale`, W2/c is folded into Sel, BN scales
    into the trunk weights.  DMA is compact (1.35MB/core, b0-first order).
  * psum banks: zp 2x3 (z-chunks, 6 per tile, one strided-AP Silu each) +
    wp 1 (w accumulator) + gx 1 (G/V/trunk rotating) = 8.
"""

from contextlib import ExitStack

import ml_dtypes
import numpy as np

import concourse.bacc as bacc
import concourse.tile as tile
from concourse import mybir
from concourse.bass_utils import run_bass_kernel_spmd

F32 = mybir.dt.float32
BF16 = mybir.dt.bfloat16
BF16NP = ml_dtypes.bfloat16

B, T, TQ, E = 32, 200, 50, 64
P, C = 128, 64
U = 36
NCORES = 8
BL = B // NCORES   # batches per core
QU = TQ * U        # 1800
CH = 120           # qu-rows per mm1 chunk
NCH = QU // CH     # 15 chunks per batch
EPS = 1e-6

TCH = [(0, 128), (128, 72)]

# z-chunk column offsets inside a [128, 1536] (3-bank) psum tile, ordered so
# any prefix of 3 and the full 6 form rectangular strided APs:
#   first 3: stride 512; 6: [2 x stride 200, 3 x stride 512]
SLOT = [0, 512, 1024, 200, 712, 1224]

# (chunk_start, n_chunks) silu tiles per batch: a small 3-chunk first tile
# lets the next batch's first silu fire with zero ACT gap at batch borders
# (the explicit schedule below references these splits directly)
TILES = [(0, 3), (3, 6), (9, 6)]

# consts pack layout (columns of one [128, 1232] bf16 tensor)
CB0 = 0            # trunk weights: w1f k0 | w1f k1 | w2f k0 | w2f k1 | w3f
CH1 = 832          # chunk1 = [up^T[64:128]; cx^T] broadcast over q (200 cols)
CH0 = 1032         # chunk0 = [zeros(interest); up^T[0:64]] broadcast (200)
NCONST = 1232

BW = T + QU        # augLR cols per batch: [augL_b (200) | augR_b (1800)]

_CACHE = {}


def _build_program():
    nc = bacc.Bacc(
        "TRN2", target_bir_lowering=False, debug=False, num_devices=NCORES
    )
    d_augLR = nc.declare_dram_parameter(
        "augLR", [65, BL * BW], BF16, isOutput=False
    )
    d_sel = nc.declare_dram_parameter("sel", [CH, NCH * 4], BF16, isOutput=False)
    d_ubt = nc.declare_dram_parameter("ubt", [128, 2 * BL * E], BF16, isOutput=False)
    d_consts = nc.declare_dram_parameter("consts", [128, NCONST], BF16, isOutput=False)
    d_out = nc.declare_dram_parameter("out", [64, BL * TQ], F32, isOutput=True)

    c_dice = float(1.0 / np.sqrt(1.0 + EPS))

    with tile.TileContext(nc) as tc:
        with ExitStack() as ctx:
            singles = ctx.enter_context(tc.tile_pool(name="singles", bufs=1))
            work = ctx.enter_context(tc.tile_pool(name="work", bufs=2))
            ps_z = ctx.enter_context(tc.tile_pool(name="ps_z", bufs=2, space="PSUM"))
            ps_w = ctx.enter_context(tc.tile_pool(name="ps_w", bufs=1, space="PSUM"))
            ps_g = ctx.enter_context(tc.tile_pool(name="ps_g", bufs=1, space="PSUM"))

            augLR = singles.tile([65, BL * BW], BF16)
            # first DMA: b0's augL + augR chunks 0-2 -> unblocks the first
            # mm1 tile + silu as early as possible
            nc.sync.dma_start(
                out=augLR[:, 0:T + 3 * CH], in_=d_augLR[:, 0:T + 3 * CH]
            )
            nc.sync.dma_start(
                out=augLR[:, T + 3 * CH:T + 9 * CH],
                in_=d_augLR[:, T + 3 * CH:T + 9 * CH],
            )
            nc.sync.dma_start(
                out=augLR[:, T + 9 * CH:BW], in_=d_augLR[:, T + 9 * CH:BW]
            )
            sel = singles.tile([CH, NCH * 4], BF16)
            nc.sync.dma_start(out=sel, in_=d_sel[:])
            nc.sync.dma_start(out=augLR[:, BW:2 * BW], in_=d_augLR[:, BW:2 * BW])
            ubt = singles.tile([128, 2 * BL * E], BF16)
            nc.sync.dma_start(out=ubt, in_=d_ubt[:])
            consts = singles.tile([128, NCONST], BF16)
            nc.sync.dma_start(out=consts, in_=d_consts[:])
            nc.sync.dma_start(
                out=augLR[:, 2 * BW:3 * BW], in_=d_augLR[:, 2 * BW:3 * BW]
            )
            nc.sync.dma_start(
                out=augLR[:, 3 * BW:], in_=d_augLR[:, 3 * BW:]
            )

            w1f_sb = [consts[:, 0:256], consts[:, 256:512]]
            w2f_sb = [consts[:, 512:640], consts[:, 640:768]]
            w3f_sb = consts[:, 768:832]
            chunk1 = consts[:, CH1:CH1 + BL * TQ]
            chunk0 = consts[:, CH0:CH0 + BL * TQ]

            w_tiles = {}
            wps = {}
            # s' for ALL batches lives in one tensor, columns indexed by the
            # GLOBAL chunk id g = 15b + c -- contiguous across batch borders,
            # so silu tiles can span two batches (no border ACT gaps).
            s_all = singles.tile([128, BL * NCH * T], BF16, name="s_all")

            def mm1_tile(g0, ncb):
                """z^T global chunks g0..g0+ncb -> fresh zp tile."""
                zp = ps_z.tile([128, 1536], F32, tag="zp")
                for i in range(ncb):
                    g = g0 + i
                    b, c = g // NCH, g % NCH
                    nc.tensor.matmul(
                        zp[0:CH, SLOT[i]:SLOT[i] + T],
                        augLR[:, b * BW + T + CH * c:b * BW + T + CH * (c + 1)],
                        augLR[:, b * BW:b * BW + T],
                        start=True,
                        stop=True,
                    )
                return zp

            def silu_tile(g0, ncb, zp):
                """One Silu draining ncb z-chunks (ncb in {3, 6})."""
                zin = zp[0:CH, :].rearrange("p (a r) -> p a r", r=512)[:, :, 0:400]
                zin = zin.rearrange("p a (two c) -> p two a c", two=2)
                out = s_all[0:CH, T * g0:T * (g0 + ncb)]
                if ncb == 6:
                    nc.scalar.activation(
                        out.rearrange("p (two a c) -> p two a c", two=2, c=T),
                        zin,
                        mybir.ActivationFunctionType.Silu,
                        scale=c_dice,
                    )
                else:
                    assert ncb == 3
                    nc.scalar.activation(
                        out.rearrange("p (a c) -> p a c", c=T),
                        zin[:, 0],
                        mybir.ActivationFunctionType.Silu,
                        scale=c_dice,
                    )

            def wsel_init(b):
                """Allocate + zero wp for batch b.  Called explicitly ahead
                of the first wsel so the memset clears the DVE queue early
                instead of gating the batch's first wsel matmuls."""
                if b not in wps:
                    wps[b] = ps_w.tile([128, 128], F32, tag="wp", name=f"wp{b}")
                    nc.vector.memset(wps[b][:], 0.0)

            def wsel_tile(b, t0c, ncb, tch_only=None):
                """Accumulate chunks' banded W2-contraction into wp.  Each
                chunk's Sel band is 4 columns wide (its q-range), so matmuls
                are 4-col with overlapping accumulation windows.  wp is
                pre-zeroed by DVE memset and every matmul uses start=False
                (clean accumulate semantics, no bank pending-zero games)."""
                wsel_init(b)
                wp = wps[b]
                for tch, (tc0, tlen) in enumerate(TCH):
                    if tch_only is not None and tch != tch_only:
                        continue
                    for i in range(ncb):
                        c = t0c + i
                        g = NCH * b + c
                        qoff = (10 * c) // 3
                        nc.tensor.matmul(
                            wp[0:tlen, 64 * tch + qoff:64 * tch + qoff + 4],
                            s_all[0:CH, T * g + tc0:T * g + tc0 + tlen],
                            sel[:, 4 * c:4 * (c + 1)],
                            start=False,
                            stop=(tch == 1 and c == NCH - 1),
                            skip_group_check=True,
                        )

            def wevict(b, q0=0, q1=TQ, split=False):
                """Evict wp cols [q0, q1) -> per-t-chunk bf16 tiles (separate
                tiles: no same-tile WAW between the two copies)."""
                if b not in w_tiles:
                    w_tiles[b] = [
                        work.tile([128, 64], BF16, tag=f"wA{b % 2}",
                                  name=f"wA{b}"),
                        work.tile([128, 64], BF16, tag=f"wB{b % 2}",
                                  name=f"wB{b}"),
                    ]
                wp = wps[b]
                nc.vector.tensor_copy(
                    w_tiles[b][0][:, q0:q1], wp[:, q0:q1]
                )
                if split:  # tail: second copy on the idle ACT engine
                    nc.scalar.copy(
                        w_tiles[b][1][0:72, q0:q1], wp[0:72, 64 + q0:64 + q1]
                    )
                else:
                    nc.vector.tensor_copy(
                        w_tiles[b][1][0:72, q0:q1], wp[0:72, 64 + q0:64 + q1]
                    )

            def g_pair(pb):
                """interest for pair (2pb, 2pb+1), pair-packed [128, 50]."""
                gp = ps_g.tile([128, 512], F32, tag="gx")
                for half in range(2):
                    b = 2 * pb + half
                    for tch, (tc0, tlen) in enumerate(TCH):
                        nc.tensor.matmul(
                            gp[64 * half:64 * half + 64, 0:TQ],
                            ubt[0:tlen, tch * BL * E + b * E:tch * BL * E + (b + 1) * E],
                            w_tiles[b][tch][0:tlen, 0:TQ],
                            start=(tch == 0),
                            stop=(tch == 1),
                        )
                pair = (2 * pb, 2 * pb + 1)
                nc.vector.tensor_copy(
                    chunk0[0:64, pair[0] * TQ:(pair[0] + 1) * TQ], gp[0:64, 0:TQ]
                )
                nc.vector.tensor_copy(
                    chunk0[0:64, pair[1] * TQ:(pair[1] + 1) * TQ], gp[64:128, 0:TQ]
                )

            v_sb = {}

            def v_precompute():
                """V = ub @ w1f[0:64] for pair-1 batches (2, 3): folds the
                interest matmul (G) into the trunk's first layer so the tail
                chain skips G + g_fin entirely.  Runs mid-stream (off the
                critical path); stationary is augL rows 0:64.  Both batches
                share one psum tile and one wide DVE eviction per t-chunk."""
                for tch, (tc0, tlen) in enumerate(TCH):
                    v_sb[tch] = singles.tile([128, 512], BF16, name=f"v{tch}")
                    # wp bank is free between wevict(1) and wsel(2,..):
                    # using it here keeps the gx chain (g_pair->trunk0) short
                    vp = ps_w.tile([128, 512], F32, tag="wp")
                    for b in (2, 3):
                        # single start: b3's region rides b2's bank-wide
                        # pending-zero (same trick as wsel_tile)
                        nc.tensor.matmul(
                            vp[0:tlen, (b - 2) * 256:(b - 1) * 256],
                            augLR[0:64, b * BW + tc0:b * BW + tc0 + tlen],
                            consts[0:64, 0:256],
                            start=(b == 2),
                            stop=(b == 3),
                            skip_group_check=True,
                        )
                    nc.vector.tensor_copy(
                        v_sb[tch][0:tlen, :], vp[0:tlen, :]
                    )

            tail_xp = {}

            def trunk_tail_waveA(pb):
                """Tail trunk, early wave (runs during the last silu): G is
                folded into layer 1 via x1 = V^T @ w + W1f[64:]^T @ [up;cx].
                Everything except b_last's q-40:50 w-columns is ready once
                wsel of chunks 0-11 lands, so layer-1 is mostly done before
                the final silu finishes.  psum: mch0 in gx, mch1 in a freed
                zp slot (wp slot still accumulates b_last's wsel)."""
                n0c = 2 * pb * TQ
                cols = slice(n0c, n0c + 2 * TQ)
                for mch in range(2):
                    if mch == 1:
                        xp = ps_z.tile([128, 1536], F32, tag="zp")
                    else:
                        xp = ps_g.tile([128, 512], F32, tag="gx")
                    tail_xp[mch] = xp
                    nc.tensor.matmul(
                        xp[:, 0:2 * TQ],
                        w1f_sb[0][64:128, mch * 128:(mch + 1) * 128],
                        chunk0[64:128, cols], start=True, stop=False,
                    )
                    nc.tensor.matmul(
                        xp[:, 0:2 * TQ],
                        w1f_sb[1][:, mch * 128:(mch + 1) * 128],
                        chunk1[:, cols], start=False, stop=False,
                    )
                    for half in range(2):
                        b = 2 * pb + half
                        qe = 40 if half == 1 else TQ  # b_last: A-cols only
                        for tch, (tc0, tlen) in enumerate(TCH):
                            nc.tensor.matmul(
                                xp[:, half * TQ:half * TQ + qe],
                                v_sb[tch][0:tlen,
                                          half * 256 + mch * 128:
                                          half * 256 + (mch + 1) * 128],
                                w_tiles[b][tch][0:tlen, 0:qe],
                                start=False,
                                stop=False,
                                skip_group_check=True,
                            )

            def trunk_tail_waveB(pb):
                """Tail trunk, late wave: only b_last's w[:, 40:50] columns
                remained; then relu -> layer2 -> layer3 -> out DMAs."""
                n0c = 2 * pb * TQ
                b = 2 * pb + 1
                x1 = []
                for mch in range(2):
                    xp = tail_xp[mch]
                    for tch, (tc0, tlen) in enumerate(TCH):
                        nc.tensor.matmul(
                            xp[:, TQ + 40:2 * TQ],
                            v_sb[tch][0:tlen,
                                      256 + mch * 128:256 + (mch + 1) * 128],
                            w_tiles[b][tch][0:tlen, 40:TQ],
                            start=False,
                            stop=(tch == 1),
                            skip_group_check=True,
                        )
                    x1_t = work.tile([128, 2 * TQ], BF16, tag=f"x1_{mch}")
                    relu_evict(x1_t, xp[:, 0:2 * TQ], mch == 1)
                    x1.append(x1_t)

                xp2 = ps_g.tile([128, 512], F32, tag="gx")
                nc.tensor.matmul(xp2[:, 0:2 * TQ], w2f_sb[0], x1[0],
                                 start=True, stop=False)
                nc.tensor.matmul(xp2[:, 0:2 * TQ], w2f_sb[1], x1[1],
                                 start=False, stop=True)
                # x2 relu + layer 3 + out, pipelined per q-half (h0 on DVE,
                # h1 on ACT): each half's DMA overlaps the other's compute
                x2_t = work.tile([128, 2 * TQ], BF16, tag="x2")
                xp3 = ps_z.tile([64, 1536], F32, tag="zp")
                for h in range(2):
                    relu_evict(
                        x2_t[:, h * TQ:(h + 1) * TQ],
                        xp2[:, h * TQ:(h + 1) * TQ], h == 1,
                    )
                    nc.tensor.matmul(
                        xp3[:, h * TQ:(h + 1) * TQ], w3f_sb,
                        x2_t[:, h * TQ:(h + 1) * TQ], start=(h == 0),
                        stop=(h == 1), skip_group_check=True,
                    )
                    out_t = work.tile([64, TQ], F32, tag=f"outT{h}")
                    relu_evict(out_t, xp3[:, h * TQ:(h + 1) * TQ], h == 1)
                    nc.sync.dma_start(
                        out=d_out[:, n0c + h * TQ:n0c + (h + 1) * TQ],
                        in_=out_t,
                    )

            def relu_evict(out_t, xp_ap, on_act):
                if on_act:
                    nc.scalar.activation(
                        out_t, xp_ap, mybir.ActivationFunctionType.Relu
                    )
                else:
                    nc.vector.tensor_scalar_max(out_t, xp_ap, 0.0)

            def trunk_pair(pb, tail=False):
                """tail=True (last pair): second psum slot (freed wp bank) +
                ACT for half the evictions, parallelizing the stage chain."""
                n0c = 2 * pb * TQ
                cols = slice(n0c, n0c + 2 * TQ)
                x1 = []
                for mch in range(2):
                    if tail and mch == 1:
                        xp = ps_w.tile([128, 128], F32, tag="wp")
                    else:
                        xp = ps_g.tile([128, 512], F32, tag="gx")
                    nc.tensor.matmul(
                        xp[:, 0:2 * TQ],
                        w1f_sb[0][:, mch * 128:(mch + 1) * 128],
                        chunk0[:, cols], start=True, stop=False,
                    )
                    nc.tensor.matmul(
                        xp[:, 0:2 * TQ],
                        w1f_sb[1][:, mch * 128:(mch + 1) * 128],
                        chunk1[:, cols], start=False, stop=True,
                    )
                    x1_t = work.tile([128, 2 * TQ], BF16, tag=f"x1_{mch}")
                    relu_evict(x1_t, xp[:, 0:2 * TQ], tail and mch == 1)
                    x1.append(x1_t)

                xp2 = ps_g.tile([128, 512], F32, tag="gx")
                nc.tensor.matmul(xp2[:, 0:2 * TQ], w2f_sb[0], x1[0],
                                 start=True, stop=False)
                nc.tensor.matmul(xp2[:, 0:2 * TQ], w2f_sb[1], x1[1],
                                 start=False, stop=True)
                x2_t = work.tile([128, 2 * TQ], BF16, tag="x2")
                relu_evict(x2_t, xp2[:, 0:2 * TQ], False)

                if tail:
                    xp3 = ps_w.tile([64, 128], F32, tag="wp")
                else:
                    xp3 = ps_g.tile([64, 512], F32, tag="gx")
                nc.tensor.matmul(xp3[:, 0:2 * TQ], w3f_sb, x2_t,
                                 start=True, stop=True)
                out_t = work.tile([64, 2 * TQ], F32, tag="outT")
                relu_evict(out_t, xp3[:, 0:2 * TQ], tail)
                nc.sync.dma_start(out=d_out[:, cols], in_=out_t)

            def tile_step(g0, ncb):
                zp = mm1_tile(g0, ncb)
                silu_tile(g0, ncb, zp)

            # Software-pipelined schedule over GLOBAL chunk tiles
            # (3, 6x9, 3): every 6-chunk tile's mm1s fit inside the previous
            # silu's 1185ns window, so ACT runs gap-free end to end
            # (batch borders are crossed inside tiles T5=g27-32, etc.).
            tile_step(0, 3)    # b0 c0-2
            tile_step(3, 6)    # b0 c3-8
            wsel_tile(0, 0, 3)
            tile_step(9, 6)    # b0 c9-14
            wsel_tile(0, 3, 6)
            tile_step(15, 6)   # b1 c0-5
            wsel_tile(0, 9, 6)
            wevict(0)
            tile_step(21, 6)   # b1 c6-11
            wsel_tile(1, 0, 6)
            tile_step(27, 6)   # b1 c12-14 | b2 c0-2
            wsel_tile(1, 6, 6)
            tile_step(33, 6)   # b2 c3-8
            wsel_tile(1, 12, 3)
            wevict(1)
            g_pair(0)
            v_precompute()
            wsel_tile(2, 0, 3)
            trunk_pair(0)
            tile_step(39, 6)   # b2 c9-14
            wsel_tile(2, 3, 6)
            tile_step(45, 6)   # b3 c0-5
            wsel_tile(2, 9, 6)
            wevict(2)
            wsel_init(3)
            tile_step(51, 6)   # b3 c6-11
            wsel_tile(3, 0, 6)
            tile_step(57, 3)   # b3 c12-14
            wsel_tile(3, 6, 6)
            wevict(3, 0, 40)
            wsel_tile(3, 12, 3)
            trunk_tail_waveA(1)
            wevict(3, 40, TQ)
            trunk_tail_waveB(1)

    nc.compile()
    return nc


def _prepare_maps(inputs):
    f = lambda k: np.ascontiguousarray(np.asarray(inputs[k], dtype=np.float32))
    W1, W2 = f("W1"), f("W2")
    b1 = f("b1")
    Wm1, Wm2, Wm3 = f("Wm1"), f("Wm2"), f("Wm3")

    A = W1[0:64] + W1[128:192]     # q rows + (q-k) rows
    Bm = W1[64:128] - W1[128:192]  # k rows - (q-k) rows
    D = W1[192:256]                # (q*k) rows
    c = 1.0 / np.sqrt(1.0 + EPS)   # dice rsqrt(var+eps) with var=1
    cb = 1.0 / np.sqrt(1.0 + EPS)  # BN identity scale

    w1f = cb * Wm1
    w2f = cb * Wm2
    w3f = cb * Wm3
    cB = np.concatenate(
        [w1f[0:128], w1f[128:256], w2f[0:128], w2f[128:256], w3f], axis=1
    )  # (128, 832)

    # Banded Sel: chunk c's qu-rows touch exactly 4 q's starting at
    # qoff(c) = (10c)//3; sel4[p, 4c + j] = W2[u]/c for q == qoff(c)+j
    selm = np.zeros((CH, NCH * 4), np.float32)
    for cc in range(NCH):
        qoff = (10 * cc) // 3
        for p in range(CH):
            r = 120 * cc + p
            j = r // U - qoff
            selm[p, 4 * cc + j] = W2[r % U, 0] / c

    ub = f("user_behavior")        # (B, T, E)
    it = f("items")                # (B, TQ, E)
    up = f("user_profile")         # (B, P)
    cx = f("context")              # (B, C)

    in_maps = []
    for i in range(NCORES):
        s = slice(i * BL, (i + 1) * BL)
        ub_i, it_i = ub[s], it[s]

        augLR = np.empty((65, BL * BW), np.float32)
        itt = it_i.transpose(0, 2, 1)  # (BL, E, TQ)
        mprime = (
            itt[:, :, :, None] * D[None, :, None, :]
            + A[None, :, None, :]
        ).reshape(BL, E, QU)
        termq = (
            np.einsum("bqe,eu->bqu", it_i, Bm) + b1[None, None, :]
        ).reshape(BL, QU)
        for b in range(BL):
            augLR[0:64, b * BW:b * BW + T] = ub_i[b].T
            augLR[64, b * BW:b * BW + T] = 1.0
            cols = slice(b * BW + T, (b + 1) * BW)
            augLR[0:64, cols] = mprime[b]
            augLR[64, cols] = termq[b]

        ubt = np.zeros((128, 2 * BL * E), np.float32)
        for b in range(BL):
            ubt[0:128, b * E:(b + 1) * E] = ub_i[b, 0:128]
            ubt[0:72, BL * E + b * E:BL * E + (b + 1) * E] = ub_i[b, 128:200]

        consts = np.zeros((128, NCONST), np.float32)
        consts[:, 0:832] = cB
        for b in range(BL):
            cols = slice(CH1 + b * TQ, CH1 + (b + 1) * TQ)
            consts[0:64, cols] = up[s][b, 64:128, None]
            consts[64:128, cols] = cx[s][b, :, None]
            cols = slice(CH0 + b * TQ, CH0 + (b + 1) * TQ)
            consts[64:128, cols] = up[s][b, 0:64, None]

        in_maps.append({
            "augLR": np.ascontiguousarray(augLR.astype(BF16NP)),
            "sel": np.ascontiguousarray(selm.astype(BF16NP)),
            "ubt": np.ascontiguousarray(ubt.astype(BF16NP)),
            "consts": np.ascontiguousarray(consts.astype(BF16NP)),
        })
    return in_maps


def run(inputs, trace=False):
    if "nc" not in _CACHE:
        _CACHE["nc"] = _build_program()
    nc = _CACHE["nc"]
    in_maps = _prepare_maps(inputs)
    res = run_bass_kernel_spmd(nc, in_maps, list(range(NCORES)), trace=trace)
    out = np.empty((B, TQ, 64), dtype=np.float32)
    for i in range(NCORES):
        out[i * BL:(i + 1) * BL] = (
            res.results[i]["out"].T.reshape(BL, TQ, 64)
        )
    return out, res


def kernel(**inputs):
    out, _ = run(inputs, trace=False)
    return out
